# revision 5
# baseline (speedup 1.0000x reference)
"""Trainium2 Bass kernel for nn_CrossAttentionS2T.

Sharding: data-parallel over batch B=8 across the 8 NeuronCores (one batch
element per core); small weights replicated.

Per-core pipeline (all matmuls bf16 with f32 PSUM accumulation):
  A) build X^T=[d,iq] and S^T=[d,ik] (bf16) via DVE cast/add + PE transpose.
     S = rearrange(s_x,'n t d -> (t n) d') + pos is done by segmented DMA + add.
  B) Q^T = WqT.T @ X^T, K^T = WkT.T @ S^T (transposed outputs, per-partition
     bias on eviction), V = S @ Wv^T (natural layout, bias via K=1 ones-matmul),
     V_aug = [V | 1] (ones column -> softmax denominator for free).
  C) per head pair: scores^T chunk = K_h @ Q_h^T (two heads row-packed in the
     PE array, K=64 each), exp via one ScalarE ACTIVATE over both heads'
     PSUM banks (scale=1/sqrt(hd) folded in), AV: out^T[65,iq] accumulated
     with V_aug stationary.  Row 64 of out^T is the softmax denominator.
     1/sum via partition-broadcast DMA + reciprocal_approx_fast, applied to
     the bf16 A^T tiles.
  D) final = A^T.T @ Wp^T + b (natural layout), DMA out.
"""

import numpy as np
import ml_dtypes

P = 128
D = 768
H = 12
HD = 64
NQ = 1576
NK = 1576
T = 8
NS = 197
DC = 6                      # d chunks of 128
IQC = 394                   # iq chunk (4 chunks exactly)
N_IQC = 4
CHUNKS = [(c * P, P) for c in range(12)] + [(1536, 40)]   # ik / iq row chunks
BF16 = ml_dtypes.bfloat16

_CACHE = {}


def _build_program():
    from contextlib import ExitStack
    import concourse.bass as bass
    import concourse.tile as tile
    from concourse import bacc, mybir
    from concourse.masks import make_identity

    f32 = mybir.dt.float32
    bf16 = mybir.dt.bfloat16
    EXP = mybir.ActivationFunctionType.Exp

    nc = bacc.Bacc("TRN2", target_bir_lowering=False, debug=False,
                   enable_asserts=False, num_devices=8)

    s_x_b = nc.dram_tensor("s_x_b", [NS, T, D], f32, kind="ExternalInput").ap()
    t_x_b = nc.dram_tensor("t_x_b", [NQ, D], f32, kind="ExternalInput").ap()
    pos = nc.dram_tensor("pos", [NK, D], f32, kind="ExternalInput").ap()
    wqT = nc.dram_tensor("wqT", [D, D], bf16, kind="ExternalInput").ap()
    wkvT = nc.dram_tensor("wkvT", [D, 2 * D], bf16, kind="ExternalInput").ap()
    wpT = nc.dram_tensor("wpT", [D, D], bf16, kind="ExternalInput").ap()
    qb_d = nc.dram_tensor("qb", [D], f32, kind="ExternalInput").ap()
    kvb_d = nc.dram_tensor("kvb", [2 * D], f32, kind="ExternalInput").ap()
    pjb_d = nc.dram_tensor("pjb", [D], f32, kind="ExternalInput").ap()
    out_d = nc.dram_tensor("out", [NQ, D], f32, kind="ExternalOutput").ap()
    sums_scr = nc.dram_tensor("sums_scr", [DC, 2, NQ], f32, kind="Internal").ap()

    with tile.TileContext(nc) as tc, ExitStack() as ctx:
        persist = ctx.enter_context(tc.tile_pool(name="persist", bufs=1))
        pspool = ctx.enter_context(tc.tile_pool(name="ps", bufs=4, space="PSUM"))

        # ---- constants / weights ----
        ident = persist.tile([P, P], bf16, tag="ident")
        make_identity(nc, ident)
        ones_bf = persist.tile([1, P], bf16, tag="ones")
        nc.vector.memset(ones_bf, 1.0)

        wq_sb = persist.tile([P, DC, D], bf16, tag="wq")
        nc.sync.dma_start(wq_sb, wqT.rearrange("(c p) o -> p c o", p=P))
        wkv_sb = persist.tile([P, DC, 2 * D], bf16, tag="wkv")
        nc.sync.dma_start(wkv_sb, wkvT.rearrange("(c p) o -> p c o", p=P))
        wp_sb = persist.tile([P, DC, D], bf16, tag="wp")
        nc.sync.dma_start(wp_sb, wpT.rearrange("(c p) o -> p c o", p=P))

        qb_sb = persist.tile([P, DC], f32, tag="qb")
        nc.sync.dma_start(qb_sb, qb_d.rearrange("(c p) -> p c", p=P))
        kb_sb = persist.tile([P, DC], f32, tag="kb")
        nc.sync.dma_start(kb_sb, kvb_d[0:D].rearrange("(c p) -> p c", p=P))
        kvb_row = persist.tile([1, 2 * D], f32, tag="kvbrow")
        nc.sync.dma_start(kvb_row, kvb_d.rearrange("(a x) -> a x", a=1))
        kvb_bf = persist.tile([1, 2 * D], bf16, tag="kvbbf")
        nc.vector.tensor_copy(kvb_bf, kvb_row)
        pjb_row = persist.tile([1, D], f32, tag="pjbrow")
        nc.sync.dma_start(pjb_row, pjb_d.rearrange("(a x) -> a x", a=1))
        pjb_bf = persist.tile([1, D], bf16, tag="pjbbf")
        nc.vector.tensor_copy(pjb_bf, pjb_row)

        QT = persist.tile([P, DC, NQ], bf16, tag="QT")
        KT = persist.tile([P, DC, NK], bf16, tag="KT")
        Vaug = persist.tile([P, len(CHUNKS), H, HD + 1], bf16, tag="Vaug")
        sums_st = persist.tile([65, 2, NQ], f32, tag="sums")

        # ---- phase A: X^T and S^T ----
        with tc.tile_pool(name="phA", bufs=1) as pha:
            XT = pha.tile([P, DC, NQ], bf16, tag="XT")
            ST = pha.tile([P, DC, NK], bf16, tag="ST")

            def transpose_to(dst, src_bf, r0, mc):
                # src_bf: [mc, D] bf16 -> dst[:, :, r0:r0+mc] ([128, DC, *])
                for g in range(2):
                    tp = pspool.tile([P, 3, P], bf16, tag="ps")
                    for j in range(3):
                        dc = 3 * g + j
                        nc.tensor.transpose(
                            tp[:, j, 0:mc],
                            src_bf[0:mc, dc * P:(dc + 1) * P],
                            ident[0:mc, 0:mc],
                        )
                    nc.vector.tensor_copy(dst[:, 3 * g:3 * g + 3, r0:r0 + mc],
                                          tp[:, :, 0:mc])

            for (r0, mc) in CHUNKS:
                x_st = pha.tile([P, D], f32, tag="xst", bufs=3)
                nc.sync.dma_start(x_st[0:mc, :], t_x_b[r0:r0 + mc, :])
                x_bf = pha.tile([P, D], bf16, tag="xbf", bufs=3)
                nc.vector.tensor_copy(x_bf[0:mc, :], x_st[0:mc, :])
                transpose_to(XT, x_bf, r0, mc)

            for (r0, mc) in CHUNKS:
                s_st = pha.tile([P, D], f32, tag="sst", bufs=3)
                r = r0
                while r < r0 + mc:
                    t0, n0 = divmod(r, NS)
                    take = min(r0 + mc - r, NS - n0)
                    nc.sync.dma_start(s_st[r - r0:r - r0 + take, :],
                                      s_x_b[n0:n0 + take, t0, :])
                    r += take
                p_st = pha.tile([P, D], f32, tag="pst", bufs=3)
                nc.sync.dma_start(p_st[0:mc, :], pos[r0:r0 + mc, :])
                s_bf = pha.tile([P, D], bf16, tag="sbf", bufs=3)
                nc.vector.tensor_add(s_bf[0:mc, :], s_st[0:mc, :], p_st[0:mc, :])
                transpose_to(ST, s_bf, r0, mc)

            # ---- phase B: projections ----
            # Q^T and K^T: [o(128-part), iq] accumulated over 6 d-chunks.
            for (dst, w_off, bias) in ((QT, None, qb_sb), (KT, 0, kb_sb)):
                for oc in range(DC):
                    for half in range(2):
                        q0 = half * 788
                        ps = pspool.tile([P, 788], f32, tag="ps")
                        for kc in range(DC):
                            if dst is QT:
                                lhsT = wq_sb[:, kc, oc * P:(oc + 1) * P]
                            else:
                                lhsT = wkv_sb[:, kc, oc * P:(oc + 1) * P]
                            src = XT if dst is QT else ST
                            st = (kc == 0)
                            sp = (kc == DC - 1)
                            nc.tensor.matmul(ps[:, 0:512], lhsT=lhsT,
                                             rhs=src[:, kc, q0:q0 + 512],
                                             start=st, stop=sp)
                            nc.tensor.matmul(ps[:, 512:788], lhsT=lhsT,
                                             rhs=src[:, kc, q0 + 512:q0 + 788],
                                             start=st, stop=sp)
                        nc.vector.tensor_scalar_add(dst[:, oc, q0:q0 + 788],
                                                    ps, bias[:, oc:oc + 1])

            # V natural [ik, o] with bias via K=1 ones-matmul; evict into Vaug.
            for ci, (r0, mc) in enumerate(CHUNKS):
                ps = pspool.tile([P, D], f32, tag="ps")
                for kc in range(DC):
                    st = (kc == 0)
                    nc.tensor.matmul(ps[0:mc, 0:512],
                                     lhsT=ST[:, kc, r0:r0 + mc],
                                     rhs=wkv_sb[:, kc, D:D + 512],
                                     start=st, stop=False)
                    nc.tensor.matmul(ps[0:mc, 512:768],
                                     lhsT=ST[:, kc, r0:r0 + mc],
                                     rhs=wkv_sb[:, kc, D + 512:2 * D],
                                     start=st, stop=False)
                nc.tensor.matmul(ps[0:mc, 0:512], lhsT=ones_bf[0:1, 0:mc],
                                 rhs=kvb_bf[0:1, D:D + 512],
                                 start=False, stop=True)
                nc.tensor.matmul(ps[0:mc, 512:768], lhsT=ones_bf[0:1, 0:mc],
                                 rhs=kvb_bf[0:1, D + 512:2 * D],
                                 start=False, stop=True)
                nc.vector.tensor_copy(
                    Vaug[0:mc, ci, :, 0:HD],
                    ps[0:mc, :].rearrange("p (h e) -> p h e", h=H))
                nc.vector.memset(Vaug[:, ci, :, HD:HD + 1], 1.0)

        # ---- phase C: attention, head pairs ----
        with tc.tile_pool(name="phC", bufs=1) as phc:
            AT = phc.tile([P, DC, NQ], bf16, tag="AT")
            scale = float(HD) ** -0.5
            for pr in range(DC):
                for iqc in range(N_IQC):
                    q0 = iqc * IQC
                    avA = pspool.tile([65, IQC], f32, tag="ps")
                    avB = pspool.tile([65, IQC], f32, tag="ps")
                    for ci, (r0, mc) in enumerate(CHUNKS):
                        sc = pspool.tile([P, 1024], f32, tag="ps")
                        nc.tensor.matmul(sc[0:mc, 0:IQC],
                                         lhsT=KT[0:64, pr, r0:r0 + mc],
                                         rhs=QT[0:64, pr, q0:q0 + IQC],
                                         start=True, stop=True)
                        nc.tensor.matmul(sc[0:mc, 512:512 + IQC],
                                         lhsT=KT[64:128, pr, r0:r0 + mc],
                                         rhs=QT[64:128, pr, q0:q0 + IQC],
                                         start=True, stop=True)
                        pt = phc.tile([P, 2, IQC], bf16, tag="pt", bufs=3)
                        sc_v = sc[0:mc, :].rearrange("p (b x) -> p b x", b=2)
                        nc.scalar.activation(pt[0:mc], sc_v[:, :, 0:IQC], EXP,
                                             scale=scale)
                        nc.tensor.matmul(avA,
                                         lhsT=Vaug[0:mc, ci, 2 * pr, :],
                                         rhs=pt[0:mc, 0, :],
                                         start=(ci == 0), stop=(ci == 12))
                        nc.tensor.matmul(avB,
                                         lhsT=Vaug[0:mc, ci, 2 * pr + 1, :],
                                         rhs=pt[0:mc, 1, :],
                                         start=(ci == 0), stop=(ci == 12))
                    for j, av in ((0, avA), (1, avB)):
                        nc.vector.tensor_copy(
                            AT[64 * j:64 * (j + 1), pr, q0:q0 + IQC],
                            av[0:64, :])
                        nc.vector.tensor_copy(sums_st[64:65, j, q0:q0 + IQC],
                                              av[64:65, :])
                rec_in = phc.tile([P, NQ], f32, tag="recin", bufs=2)
                for j in range(2):
                    # SBUF partition-stride-0 DMA doesn't lower; bounce the
                    # denominator row through DRAM, then broadcast-load it.
                    nc.sync.dma_start(sums_scr[pr, j:j + 1, :],
                                      sums_st[64:65, j, :])
                    src = bass.AP(tensor=sums_scr.tensor,
                                  offset=sums_scr.offset + (pr * 2 + j) * NQ,
                                  ap=[[0, 64], [1, NQ]])
                    nc.sync.dma_start(rec_in[64 * j:64 * (j + 1), :], src)
                recb = phc.tile([P, NQ], f32, tag="recb", bufs=2)
                nc.vector.reciprocal_approx_fast(out=recb, in_=rec_in)
                for j in range(2):
                    sl = slice(64 * j, 64 * (j + 1))
                    nc.vector.tensor_mul(AT[sl, pr, :], AT[sl, pr, :],
                                         recb[sl, :])

            # ---- phase D: output projection ----
            for (r0, mc) in CHUNKS:
                ps = pspool.tile([P, D], f32, tag="ps")
                for oc in range(DC):
                    st = (oc == 0)
                    nc.tensor.matmul(ps[0:mc, 0:512],
                                     lhsT=AT[:, oc, r0:r0 + mc],
                                     rhs=wp_sb[:, oc, 0:512],
                                     start=st, stop=False)
                    nc.tensor.matmul(ps[0:mc, 512:768],
                                     lhsT=AT[:, oc, r0:r0 + mc],
                                     rhs=wp_sb[:, oc, 512:768],
                                     start=st, stop=False)
                nc.tensor.matmul(ps[0:mc, 0:512], lhsT=ones_bf[0:1, 0:mc],
                                 rhs=pjb_bf[0:1, 0:512], start=False, stop=True)
                nc.tensor.matmul(ps[0:mc, 512:768], lhsT=ones_bf[0:1, 0:mc],
                                 rhs=pjb_bf[0:1, 512:768], start=False, stop=True)
                ost = phc.tile([P, D], f32, tag="ost", bufs=3)
                nc.vector.tensor_copy(ost[0:mc, :], ps[0:mc, :])
                nc.sync.dma_start(out_d[r0:r0 + mc, :], ost[0:mc, :])

    nc.compile()
    return nc


def _get_program():
    if "nc" not in _CACHE:
        _CACHE["nc"] = _build_program()
    return _CACHE["nc"]


def _make_in_maps(s_x, t_x, space_time_pos, q_w, q_b, kv_w, kv_b, proj_w,
                  proj_b):
    s_x = np.asarray(s_x, np.float32)
    t_x = np.asarray(t_x, np.float32)
    pos = np.ascontiguousarray(np.asarray(space_time_pos, np.float32))
    wqT = np.ascontiguousarray(np.asarray(q_w, np.float32).T).astype(BF16)
    wkvT = np.ascontiguousarray(np.asarray(kv_w, np.float32).T).astype(BF16)
    wpT = np.ascontiguousarray(np.asarray(proj_w, np.float32).T).astype(BF16)
    qb = np.ascontiguousarray(np.asarray(q_b, np.float32))
    kvb = np.ascontiguousarray(np.asarray(kv_b, np.float32))
    pjb = np.ascontiguousarray(np.asarray(proj_b, np.float32))
    in_maps = []
    for b in range(8):
        in_maps.append({
            "s_x_b": np.ascontiguousarray(s_x[:, b * T:(b + 1) * T, :]),
            "t_x_b": np.ascontiguousarray(t_x[b]),
            "pos": pos,
            "wqT": wqT, "wkvT": wkvT, "wpT": wpT,
            "qb": qb, "kvb": kvb, "pjb": pjb,
        })
    return in_maps


def run(trace=False, **inputs):
    from concourse.bass_utils import run_bass_kernel_spmd
    nc = _get_program()
    in_maps = _make_in_maps(**inputs)
    res = run_bass_kernel_spmd(nc, in_maps, core_ids=list(range(8)),
                               trace=trace)
    out = np.stack([np.asarray(res.results[b]["out"], np.float32)
                    for b in range(8)])
    return out, res


def kernel(**inputs):
    out, _ = run(trace=False, **inputs)
    return out


# revision 8
# speedup vs baseline: 333.9829x; 333.9829x over previous
"""Trainium2 Bass kernel for nn_CrossAttentionS2T.

Sharding: data-parallel over batch B=8 across the 8 NeuronCores (one batch
element per core); small weights replicated.

Per-core pipeline (all matmuls bf16 with f32 PSUM accumulation):
  A) build X^T=[d,iq] and S^T=[d,ik] (bf16) via DVE cast/add + PE transpose.
     S = rearrange(s_x,'n t d -> (t n) d') + pos is done by segmented DMA + add.
  B) Q^T = WqT.T @ X^T, K^T = WkT.T @ S^T (transposed outputs, per-partition
     bias on eviction), V = S @ Wv^T (natural layout, bias via K=1 ones-matmul),
     V_aug = [V | 1] (ones column -> softmax denominator for free).
  C) per head pair: scores^T chunk = K_h @ Q_h^T (two heads row-packed in the
     PE array, K=64 each), exp via one ScalarE ACTIVATE over both heads'
     PSUM banks (scale=1/sqrt(hd) folded in), AV: out^T[65,iq] accumulated
     with V_aug stationary.  Row 64 of out^T is the softmax denominator.
     1/sum via partition-broadcast DMA + reciprocal_approx_fast, applied to
     the bf16 A^T tiles.
  D) final = A^T.T @ Wp^T + b (natural layout), DMA out.
"""

import numpy as np
import ml_dtypes

P = 128
D = 768
H = 12
HD = 64
NQ = 1576
NK = 1576
T = 8
NS = 197
DC = 6                      # d chunks of 128
IQC = 394                   # iq chunk (4 chunks exactly)
N_IQC = 4
CHUNKS = [(c * P, P) for c in range(12)] + [(1536, 40)]   # ik / iq row chunks
BF16 = ml_dtypes.bfloat16

_CACHE = {}


def _build_program():
    from contextlib import ExitStack
    import concourse.bass as bass
    import concourse.tile as tile
    from concourse import bacc, mybir
    from concourse.masks import make_identity

    f32 = mybir.dt.float32
    bf16 = mybir.dt.bfloat16
    EXP = mybir.ActivationFunctionType.Exp

    nc = bacc.Bacc("TRN2", target_bir_lowering=False, debug=False,
                   enable_asserts=False, num_devices=8)

    s_x_b = nc.dram_tensor("s_x_b", [NS, T, D], f32, kind="ExternalInput").ap()
    t_x_b = nc.dram_tensor("t_x_b", [NQ, D], f32, kind="ExternalInput").ap()
    pos = nc.dram_tensor("pos", [NK, D], f32, kind="ExternalInput").ap()
    wqT = nc.dram_tensor("wqT", [D, D], bf16, kind="ExternalInput").ap()
    wkvT = nc.dram_tensor("wkvT", [D, 2 * D], bf16, kind="ExternalInput").ap()
    wpT = nc.dram_tensor("wpT", [D, D], bf16, kind="ExternalInput").ap()
    qb_d = nc.dram_tensor("qb", [D], f32, kind="ExternalInput").ap()
    kvb_d = nc.dram_tensor("kvb", [2 * D], f32, kind="ExternalInput").ap()
    pjb_d = nc.dram_tensor("pjb", [D], f32, kind="ExternalInput").ap()
    out_d = nc.dram_tensor("out", [NQ, D], f32, kind="ExternalOutput").ap()
    sums_scr = nc.dram_tensor("sums_scr", [DC, 2, NQ], f32, kind="Internal").ap()

    with tile.TileContext(nc) as tc, ExitStack() as ctx:
        persist = ctx.enter_context(tc.tile_pool(name="persist", bufs=1))
        pspool = ctx.enter_context(tc.tile_pool(name="ps", bufs=4, space="PSUM"))

        # ---- constants / weights ----
        ident = persist.tile([P, P], bf16, tag="ident")
        make_identity(nc, ident)
        ones_bf = persist.tile([1, P], bf16, tag="ones")
        nc.vector.memset(ones_bf, 1.0)

        wq_sb = persist.tile([P, DC, D], bf16, tag="wq")
        nc.sync.dma_start(wq_sb, wqT.rearrange("(c p) o -> p c o", p=P))
        wkv_sb = persist.tile([P, DC, 2 * D], bf16, tag="wkv")
        nc.sync.dma_start(wkv_sb, wkvT.rearrange("(c p) o -> p c o", p=P))
        wp_sb = persist.tile([P, DC, D], bf16, tag="wp")
        nc.sync.dma_start(wp_sb, wpT.rearrange("(c p) o -> p c o", p=P))

        qb_sb = persist.tile([P, DC], f32, tag="qb")
        nc.sync.dma_start(qb_sb, qb_d.rearrange("(c p) -> p c", p=P))
        kb_sb = persist.tile([P, DC], f32, tag="kb")
        nc.sync.dma_start(kb_sb, kvb_d[0:D].rearrange("(c p) -> p c", p=P))
        kvb_row = persist.tile([1, 2 * D], f32, tag="kvbrow")
        nc.sync.dma_start(kvb_row, kvb_d.rearrange("(a x) -> a x", a=1))
        kvb_bf = persist.tile([1, 2 * D], bf16, tag="kvbbf")
        nc.vector.tensor_copy(kvb_bf, kvb_row)
        pjb_row = persist.tile([1, D], f32, tag="pjbrow")
        nc.sync.dma_start(pjb_row, pjb_d.rearrange("(a x) -> a x", a=1))
        pjb_bf = persist.tile([1, D], bf16, tag="pjbbf")
        nc.vector.tensor_copy(pjb_bf, pjb_row)

        QT = persist.tile([P, DC, NQ], bf16, tag="QT")
        KT = persist.tile([P, DC, NK], bf16, tag="KT")
        Vaug = persist.tile([P, len(CHUNKS), H, HD + 1], bf16, tag="Vaug")
        sums_st = persist.tile([65, 2, NQ], f32, tag="sums")

        # ---- phase A: X^T and S^T ----
        with tc.tile_pool(name="phA", bufs=1) as pha:
            XT = pha.tile([P, DC, NQ], bf16, tag="XT")
            ST = pha.tile([P, DC, NK], bf16, tag="ST")

            def transpose_to(dst, src_bf, r0, mc):
                # src_bf: [mc, D] bf16 -> dst[:, :, r0:r0+mc] ([128, DC, *])
                for g in range(2):
                    tp = pspool.tile([P, 3, P], bf16, tag="ps")
                    for j in range(3):
                        dc = 3 * g + j
                        nc.tensor.transpose(
                            tp[:, j, 0:mc],
                            src_bf[0:mc, dc * P:(dc + 1) * P],
                            ident[0:mc, 0:mc],
                        )
                    nc.vector.tensor_copy(dst[:, 3 * g:3 * g + 3, r0:r0 + mc],
                                          tp[:, :, 0:mc])

            for (r0, mc) in CHUNKS:
                x_st = pha.tile([P, D], f32, tag="xst", bufs=3)
                nc.sync.dma_start(x_st[0:mc, :], t_x_b[r0:r0 + mc, :])
                x_bf = pha.tile([P, D], bf16, tag="xbf", bufs=3)
                nc.vector.tensor_copy(x_bf[0:mc, :], x_st[0:mc, :])
                transpose_to(XT, x_bf, r0, mc)

            for (r0, mc) in CHUNKS:
                s_st = pha.tile([P, D], f32, tag="sst", bufs=3)
                r = r0
                while r < r0 + mc:
                    t0, n0 = divmod(r, NS)
                    take = min(r0 + mc - r, NS - n0)
                    nc.sync.dma_start(s_st[r - r0:r - r0 + take, :],
                                      s_x_b[n0:n0 + take, t0, :])
                    r += take
                p_st = pha.tile([P, D], f32, tag="pst", bufs=3)
                nc.sync.dma_start(p_st[0:mc, :], pos[r0:r0 + mc, :])
                s_bf = pha.tile([P, D], bf16, tag="sbf", bufs=3)
                nc.vector.tensor_add(s_bf[0:mc, :], s_st[0:mc, :], p_st[0:mc, :])
                transpose_to(ST, s_bf, r0, mc)

            # ---- phase B: projections ----
            # Q^T and K^T: [o(128-part), iq] accumulated over 6 d-chunks.
            for (dst, w_off, bias) in ((QT, None, qb_sb), (KT, 0, kb_sb)):
                for oc in range(DC):
                    for half in range(2):
                        q0 = half * 788
                        ps = pspool.tile([P, 788], f32, tag="ps")
                        for kc in range(DC):
                            if dst is QT:
                                lhsT = wq_sb[:, kc, oc * P:(oc + 1) * P]
                            else:
                                lhsT = wkv_sb[:, kc, oc * P:(oc + 1) * P]
                            src = XT if dst is QT else ST
                            st = (kc == 0)
                            sp = (kc == DC - 1)
                            nc.tensor.matmul(ps[:, 0:512], lhsT=lhsT,
                                             rhs=src[:, kc, q0:q0 + 512],
                                             start=st, stop=sp)
                            nc.tensor.matmul(ps[:, 512:788], lhsT=lhsT,
                                             rhs=src[:, kc, q0 + 512:q0 + 788],
                                             start=st, stop=sp)
                        nc.vector.tensor_scalar_add(dst[:, oc, q0:q0 + 788],
                                                    ps, bias[:, oc:oc + 1])

            # V natural [ik, o] with bias via K=1 ones-matmul; evict into Vaug.
            for ci, (r0, mc) in enumerate(CHUNKS):
                ps = pspool.tile([P, D], f32, tag="ps")
                for kc in range(DC):
                    st = (kc == 0)
                    nc.tensor.matmul(ps[0:mc, 0:512],
                                     lhsT=ST[:, kc, r0:r0 + mc],
                                     rhs=wkv_sb[:, kc, D:D + 512],
                                     start=st, stop=False)
                    nc.tensor.matmul(ps[0:mc, 512:768],
                                     lhsT=ST[:, kc, r0:r0 + mc],
                                     rhs=wkv_sb[:, kc, D + 512:2 * D],
                                     start=st, stop=False)
                nc.tensor.matmul(ps[0:mc, 0:512], lhsT=ones_bf[0:1, 0:mc],
                                 rhs=kvb_bf[0:1, D:D + 512],
                                 start=False, stop=True)
                nc.tensor.matmul(ps[0:mc, 512:768], lhsT=ones_bf[0:1, 0:mc],
                                 rhs=kvb_bf[0:1, D + 512:2 * D],
                                 start=False, stop=True)
                nc.vector.tensor_copy(
                    Vaug[0:mc, ci, :, 0:HD],
                    ps[0:mc, :].rearrange("p (h e) -> p h e", h=H))
                nc.vector.memset(Vaug[:, ci, :, HD:HD + 1], 1.0)

        # ---- phase C: attention, head pairs ----
        with tc.tile_pool(name="phC", bufs=1) as phc:
            AT = phc.tile([P, DC, NQ], bf16, tag="AT")
            scale = float(HD) ** -0.5
            for pr in range(DC):
                for iqc in range(N_IQC):
                    q0 = iqc * IQC
                    avA = pspool.tile([65, IQC], f32, tag="ps")
                    avB = pspool.tile([65, IQC], f32, tag="ps")
                    for ci, (r0, mc) in enumerate(CHUNKS):
                        sc = pspool.tile([P, 1024], f32, tag="ps")
                        nc.tensor.matmul(sc[0:mc, 0:IQC],
                                         lhsT=KT[0:64, pr, r0:r0 + mc],
                                         rhs=QT[0:64, pr, q0:q0 + IQC],
                                         start=True, stop=True)
                        nc.tensor.matmul(sc[0:mc, 512:512 + IQC],
                                         lhsT=KT[64:128, pr, r0:r0 + mc],
                                         rhs=QT[64:128, pr, q0:q0 + IQC],
                                         start=True, stop=True)
                        pt = phc.tile([P, 2, IQC], bf16, tag="pt", bufs=3)
                        sc_v = sc[0:mc, :].rearrange("p (b x) -> p b x", b=2)
                        nc.scalar.activation(pt[0:mc], sc_v[:, :, 0:IQC], EXP,
                                             scale=scale)
                        nc.tensor.matmul(avA,
                                         lhsT=Vaug[0:mc, ci, 2 * pr, :],
                                         rhs=pt[0:mc, 0, :],
                                         start=(ci == 0), stop=(ci == 12))
                        nc.tensor.matmul(avB,
                                         lhsT=Vaug[0:mc, ci, 2 * pr + 1, :],
                                         rhs=pt[0:mc, 1, :],
                                         start=(ci == 0), stop=(ci == 12))
                    for j, av in ((0, avA), (1, avB)):
                        nc.vector.tensor_copy(
                            AT[64 * j:64 * (j + 1), pr, q0:q0 + IQC],
                            av[0:64, :])
                        nc.vector.tensor_copy(sums_st[64:65, j, q0:q0 + IQC],
                                              av[64:65, :])
                rec_in = phc.tile([P, NQ], f32, tag="recin", bufs=2)
                for j in range(2):
                    # SBUF partition-stride-0 DMA doesn't lower; bounce the
                    # denominator row through DRAM, then broadcast-load it.
                    nc.sync.dma_start(sums_scr[pr, j:j + 1, :],
                                      sums_st[64:65, j, :])
                    src = bass.AP(tensor=sums_scr.tensor,
                                  offset=sums_scr.offset + (pr * 2 + j) * NQ,
                                  ap=[[0, 64], [1, NQ]])
                    nc.sync.dma_start(rec_in[64 * j:64 * (j + 1), :], src)
                recb = phc.tile([P, NQ], f32, tag="recb", bufs=2)
                nc.vector.reciprocal_approx_fast(out=recb, in_=rec_in)
                for j in range(2):
                    sl = slice(64 * j, 64 * (j + 1))
                    nc.vector.tensor_mul(AT[sl, pr, :], AT[sl, pr, :],
                                         recb[sl, :])

            # ---- phase D: output projection ----
            for (r0, mc) in CHUNKS:
                ps = pspool.tile([P, D], f32, tag="ps")
                for oc in range(DC):
                    st = (oc == 0)
                    nc.tensor.matmul(ps[0:mc, 0:512],
                                     lhsT=AT[:, oc, r0:r0 + mc],
                                     rhs=wp_sb[:, oc, 0:512],
                                     start=st, stop=False)
                    nc.tensor.matmul(ps[0:mc, 512:768],
                                     lhsT=AT[:, oc, r0:r0 + mc],
                                     rhs=wp_sb[:, oc, 512:768],
                                     start=st, stop=False)
                nc.tensor.matmul(ps[0:mc, 0:512], lhsT=ones_bf[0:1, 0:mc],
                                 rhs=pjb_bf[0:1, 0:512], start=False, stop=True)
                nc.tensor.matmul(ps[0:mc, 512:768], lhsT=ones_bf[0:1, 0:mc],
                                 rhs=pjb_bf[0:1, 512:768], start=False, stop=True)
                ost = phc.tile([P, D], f32, tag="ost", bufs=3)
                nc.vector.tensor_copy(ost[0:mc, :], ps[0:mc, :])
                nc.sync.dma_start(out_d[r0:r0 + mc, :], ost[0:mc, :])

    nc.compile()
    return nc


def _get_program():
    if "nc" not in _CACHE:
        _CACHE["nc"] = _build_program()
    return _CACHE["nc"]


def _make_in_maps(s_x, t_x, space_time_pos, q_w, q_b, kv_w, kv_b, proj_w,
                  proj_b):
    s_x = np.asarray(s_x, np.float32)
    t_x = np.asarray(t_x, np.float32)
    pos = np.ascontiguousarray(np.asarray(space_time_pos, np.float32))
    wqT = np.ascontiguousarray(np.asarray(q_w, np.float32).T).astype(BF16)
    wkvT = np.ascontiguousarray(np.asarray(kv_w, np.float32).T).astype(BF16)
    wpT = np.ascontiguousarray(np.asarray(proj_w, np.float32).T).astype(BF16)
    qb = np.ascontiguousarray(np.asarray(q_b, np.float32))
    kvb = np.ascontiguousarray(np.asarray(kv_b, np.float32))
    pjb = np.ascontiguousarray(np.asarray(proj_b, np.float32))
    in_maps = []
    for b in range(8):
        in_maps.append({
            "s_x_b": np.ascontiguousarray(s_x[:, b * T:(b + 1) * T, :]),
            "t_x_b": np.ascontiguousarray(t_x[b]),
            "pos": pos,
            "wqT": wqT, "wkvT": wkvT, "wpT": wpT,
            "qb": qb, "kvb": kvb, "pjb": pjb,
        })
    return in_maps


def run(trace=False, **inputs):
    from concourse.bass_utils import run_bass_kernel_spmd
    nc = _get_program()
    in_maps = _make_in_maps(**inputs)
    res = run_bass_kernel_spmd(nc, in_maps, core_ids=list(range(8)),
                               trace=trace)
    out = np.stack([np.asarray(res.results[b]["out"], np.float32)
                    for b in range(8)])
    return out, res


def kernel(**inputs):
    out, _ = run(trace=False, **inputs)
    return out


def make_runner(**inputs):
    """Persistent jitted runner with device-resident inputs, for timing.

    Returns call_n(n) -> seconds-per-execution (n async launches, one block).
    """
    import jax
    from jax.experimental.shard_map import shard_map
    from jax.sharding import Mesh, PartitionSpec, NamedSharding
    from concourse import mybir
    import concourse.bass2jax as b2j

    nc = _get_program()
    b2j.install_neuronx_cc_hook()
    in_maps = _make_in_maps(**inputs)
    n_cores = 8
    in_names, out_names, out_avals, zero_outs = [], [], [], []
    part_name = (nc.partition_id_tensor.name
                 if nc.partition_id_tensor else None)
    for alloc in nc.m.functions[0].allocations:
        if not isinstance(alloc, mybir.MemoryLocationSet):
            continue
        name = alloc.memorylocations[0].name
        if alloc.kind == "ExternalInput":
            if name != part_name:
                in_names.append(name)
        elif alloc.kind == "ExternalOutput":
            out_names.append(name)
            shape = tuple(alloc.tensor_shape)
            dtype = mybir.dt.np(alloc.dtype)
            out_avals.append(jax.core.ShapedArray(shape, dtype))
            zero_outs.append(np.zeros(shape, dtype))
    n_params = len(in_names)
    bind_names = in_names + out_names
    if part_name is not None:
        bind_names = bind_names + [part_name]
    bind_names = tuple(bind_names)

    def _body(*args):
        operands = list(args)
        if part_name is not None:
            operands.append(b2j.partition_id_tensor())
        outs = b2j._bass_exec_p.bind(
            *operands, out_avals=tuple(out_avals), in_names=bind_names,
            out_names=tuple(out_names), lowering_input_output_aliases=(),
            sim_require_finite=True, sim_require_nnan=True, nc=nc)
        return tuple(outs)

    devices = jax.devices()[:n_cores]
    mesh = Mesh(np.asarray(devices), ("core",))
    spec = PartitionSpec("core")
    n_args = n_params + len(out_names)
    sharded = jax.jit(
        shard_map(_body, mesh=mesh, in_specs=(spec,) * n_args,
                  out_specs=(spec,) * len(out_names), check_rep=False),
        keep_unused=True)
    concat_in = [np.concatenate([np.asarray(in_maps[c][nm])
                                 for c in range(n_cores)], axis=0)
                 for nm in in_names]
    concat_zeros = [np.zeros((n_cores * z.shape[0], *z.shape[1:]), z.dtype)
                    for z in zero_outs]
    sh = NamedSharding(mesh, spec)
    dev_args = [jax.device_put(a, sh) for a in concat_in + concat_zeros]

    def call_n(n=1):
        import time
        outs = sharded(*dev_args)
        jax.block_until_ready(outs)  # warm / compile
        t0 = time.time()
        res = None
        for _ in range(n):
            res = sharded(*dev_args)
        jax.block_until_ready(res)
        return (time.time() - t0) / n

    return call_n
